# revision 1
# baseline (speedup 1.0000x reference)
"""Grouped GEMM (MoE routing) Trainium2 kernel.

Problem: x [32768, 2048] bf16, tokens pre-grouped into E=8 contiguous
segments; weights [8, 2048, 1024] bf16; splits_cpu [8] int32 segment
sizes. out[seg_e] = x[seg_e] @ weights[e], fp32 accumulation, bf16 out.

Strategy: expert-parallel over 8 NeuronCores. Core e gets its expert's
token segment (host-sliced, host-transposed to K-major tiles) plus
weights[e], and runs a dense 4096x2048x1024 matmul:
  - w (4 MiB) cached fully in SBUF; x streamed in 32 m-tiles of
    [128k x 16ko x 128tok] (512 KiB each), interleaved x/w DMA issue on
    the sync hardware queue (x0, w0, x1, w1, ..., w4..w15, x4...).
  - HAM warmup: 11 dummy matmuls fed by a DVE memset (vector's preamble
    retires early) keep the PE busy from ~7.7 us so the 2.4 GHz
    clock-gate opens (~3.4 us busy window) right as real data lands
    (~12.3 us median; the warmup span must also absorb the hole before
    x1 arrives or a HAM window drops to half clock for 3.4 us).
  - first block: 4 m-tiles x 16 ko on 8 PSUM banks, matmuls emitted in
    DMA-arrival order (pairs sorted by max(x_mo, w_ko) arrival) so the
    PE chases the incoming stream without waiting for x3/w3 the way a
    strict ko-major round order would.
  - steady state: per m-tile, 2 PSUM banks (N=512 each), 16-step K
    accumulation, PSUM -> bf16 SBUF copy on ACT/DVE, DMA out on sync.
  - tail: last m-tile runs n-major (16 ko for cols 0:512, evict+DMA
    while cols 512:1024 run), and the final half evict is split into
    two 256-col copies on scalar+vector with DMAs on two queues, so
    only ~0.4 us of copy trails the final matmul.
Compute bound: 1024 matmuls of 128x128x512 per core; steady spacing is
N/2.4GHz + ~2.5ns NX overhead = ~216 ns -> ~221 us stream.
"""

import numpy as np

P = 128
E = 8
K_DIM = 2048
N_DIM = 1024
KO_TILES = K_DIM // P  # 16
WARMUP_MMS = 11

_CACHE = {}


def _block_pair_order(block, ko_tiles):
    """(mo, ko) pairs sorted by estimated DMA arrival time.

    Single sync hw queue, issue order x0,w0,x1,w1,...,w4..; per-queue
    service ~1.36 us per x tile, ~0.69 per w chunk. The absolute
    values only shape the emission order.
    """
    ax = {}
    aw = {}
    t = 0.0
    for i in range(block):
        t += 1.36
        ax[i] = t
        t += 0.69
        aw[i] = t
    for ko in range(block, ko_tiles):
        t += 0.69
        aw[ko] = t
    pairs = [(mo, ko) for mo in range(block) for ko in range(ko_tiles)]
    pairs.sort(key=lambda p: (max(ax[p[0]], aw[p[1]]), p[1], p[0]))
    return pairs


def _build(mo_tiles):
    """Build + bacc-compile the per-core Bass program for mo_tiles m-tiles."""
    import concourse.mybir as mybir
    import concourse.tile as tile
    from concourse import bacc

    nc = bacc.Bacc("TRN2", target_bir_lowering=False, debug=False)
    dt = mybir.dt.bfloat16
    f32 = mybir.dt.float32

    # xt[mo, p, ko, mi] = x_seg[mo*128 + mi, ko*128 + p]
    xt = nc.dram_tensor("xt", [mo_tiles, P, KO_TILES, P], dt, kind="ExternalInput").ap()
    # w[p, ko, n] = w_e[ko*128 + p, n]
    w = nc.dram_tensor("w", [P, KO_TILES, N_DIM], dt, kind="ExternalInput").ap()
    # out[mo, p, n] = out_seg[mo*128 + p, n]
    out = nc.dram_tensor("out", [mo_tiles, P, N_DIM], dt, kind="ExternalOutput").ap()

    BLOCK = min(4, mo_tiles)

    with tile.TileContext(nc) as tc:
        with (
            tc.tile_pool(name="const", bufs=1) as cpool,
            tc.tile_pool(name="wpool", bufs=1) as wpool,
            tc.tile_pool(name="xpool", bufs=10) as xpool,
            tc.tile_pool(name="opool", bufs=4) as opool,
            tc.tile_pool(name="psum", bufs=8, space="PSUM") as pspool,
        ):
            # --- HAM warmup: dummy matmuls fed by a DVE memset (vector's
            # preamble retires early) keep the PE busy from ~7.7 us so the
            # 2.4 GHz clock-gate opens as real data lands (~12.3 us).
            dummy = cpool.tile([P, 640], dt)
            nc.vector.memset(dummy[:], 0.0)
            warm_ps = pspool.tile([P, 512], f32, tag="ps")
            for _ in range(WARMUP_MMS):
                nc.tensor.matmul(warm_ps[:], dummy[:, 0:P], dummy[:, P:640],
                                 start=True, stop=True)

            w_sb = wpool.tile([P, KO_TILES, N_DIM], dt)
            xq = []

            def issue_x(mo):
                t = xpool.tile([P, KO_TILES, P], dt, tag="x")
                nc.sync.dma_start(t[:], xt[mo])
                xq.append(t)

            # interleave x-tile and w-chunk loads on the sync hw queue:
            # per-queue FIFO service = block consumption order, matched to
            # the slow (8-core-contended) early delivery rate.
            issue_x(0)
            nc.sync.dma_start(w_sb[:, 0, :], w[:, 0, :])
            for ko in range(1, BLOCK):
                issue_x(ko)
                nc.sync.dma_start(w_sb[:, ko, :], w[:, ko, :])
            for ko in range(BLOCK, KO_TILES):
                nc.sync.dma_start(w_sb[:, ko, :], w[:, ko, :])

            def evict(ps0, ps1, mo):
                o_sb = opool.tile([P, N_DIM], dt, tag="o")
                nc.scalar.copy(o_sb[:, 0:512], ps0[:])
                nc.vector.tensor_copy(o_sb[:, 512:1024], ps1[:])
                nc.sync.dma_start(out[mo], o_sb[:])

            # --- first block: matmul pairs in DMA-arrival order across
            # BLOCK m-tiles / 2*BLOCK PSUM banks.
            pss = [
                [
                    pspool.tile([P, 512], f32, tag="ps", name=f"ps_{mo}_{h}")
                    for h in range(2)
                ]
                for mo in range(BLOCK)
            ]
            for mo, ko in _block_pair_order(BLOCK, KO_TILES):
                first = ko == 0
                last = ko == KO_TILES - 1
                lhsT = xq[mo][:, ko, :]
                nc.tensor.matmul(pss[mo][0][:], lhsT, w_sb[:, ko, 0:512],
                                 start=first, stop=last)
                nc.tensor.matmul(pss[mo][1][:], lhsT, w_sb[:, ko, 512:1024],
                                 start=first, stop=last)
            for mo in range(BLOCK):
                evict(pss[mo][0], pss[mo][1], mo)

            # steady-state prefetches in program order; pool slots gate depth
            for mo in range(BLOCK, mo_tiles):
                issue_x(mo)

            # --- steady state: per m-tile, mo-major
            last_mo = mo_tiles - 1
            for mo in range(BLOCK, last_mo):
                x_sb = xq[mo]
                ps0 = pspool.tile([P, 512], f32, tag="ps")
                ps1 = pspool.tile([P, 512], f32, tag="ps")
                for ko in range(KO_TILES):
                    first = ko == 0
                    last = ko == KO_TILES - 1
                    lhsT = x_sb[:, ko, :]
                    nc.tensor.matmul(ps0[:], lhsT, w_sb[:, ko, 0:512],
                                     start=first, stop=last)
                    nc.tensor.matmul(ps1[:], lhsT, w_sb[:, ko, 512:1024],
                                     start=first, stop=last)
                evict(ps0, ps1, mo)

            # --- last m-tile: n-major so only a short evict trails the
            # final matmul; final half eviction split across engines.
            if last_mo >= BLOCK:
                x_sb = xq[last_mo]
                ps0 = pspool.tile([P, 512], f32, tag="ps")
                ps1 = pspool.tile([P, 512], f32, tag="ps")
                o_sb = opool.tile([P, N_DIM], dt, tag="o")
                for ko in range(KO_TILES):
                    nc.tensor.matmul(ps0[:], x_sb[:, ko, :], w_sb[:, ko, 0:512],
                                     start=ko == 0, stop=ko == KO_TILES - 1)
                nc.scalar.copy(o_sb[:, 0:512], ps0[:])
                nc.scalar.dma_start(out[last_mo][:, 0:512], o_sb[:, 0:512])
                for ko in range(KO_TILES):
                    nc.tensor.matmul(ps1[:], x_sb[:, ko, :], w_sb[:, ko, 512:1024],
                                     start=ko == 0, stop=ko == KO_TILES - 1)
                nc.scalar.copy(o_sb[:, 512:768], ps1[:, 0:256])
                nc.vector.tensor_copy(o_sb[:, 768:1024], ps1[:, 256:512])
                nc.sync.dma_start(out[last_mo][:, 512:768], o_sb[:, 512:768])
                nc.scalar.dma_start(out[last_mo][:, 768:1024], o_sb[:, 768:1024])

    nc.compile()
    return nc


def _get_nc(mo_tiles):
    if mo_tiles not in _CACHE:
        _CACHE[mo_tiles] = _build(mo_tiles)
    return _CACHE[mo_tiles]


def run(input, weights, splits_cpu, trace=False):
    import ml_dtypes
    from concourse.bass_utils import run_bass_kernel_spmd

    x = np.asarray(input)
    wts = np.asarray(weights)
    splits = [int(s) for s in np.asarray(splits_cpu)]
    assert len(splits) == E and sum(splits) == x.shape[0]
    bf16 = ml_dtypes.bfloat16

    seg_cap = max(max(splits), P)
    seg_cap = -(-seg_cap // P) * P  # round up to multiple of 128
    mo_tiles = seg_cap // P

    starts = np.cumsum([0] + splits)
    in_maps = []
    for e in range(E):
        xe = x[starts[e]:starts[e + 1]]
        if xe.shape[0] < seg_cap:
            pad = np.zeros((seg_cap - xe.shape[0], K_DIM), dtype=bf16)
            xe = np.concatenate([xe.astype(bf16), pad], axis=0)
        # [S, K] -> [mo, p, ko, mi]
        xt = np.ascontiguousarray(
            xe.astype(bf16).reshape(mo_tiles, P, KO_TILES, P).transpose(0, 3, 2, 1)
        )
        we = np.ascontiguousarray(
            wts[e].astype(bf16).reshape(KO_TILES, P, N_DIM).transpose(1, 0, 2)
        )
        in_maps.append({"xt": xt, "w": we})

    nc = _get_nc(mo_tiles)
    res = run_bass_kernel_spmd(nc, in_maps, core_ids=list(range(E)), trace=trace)

    outs = []
    for e in range(E):
        oe = np.asarray(res.results[e]["out"]).reshape(seg_cap, N_DIM)
        outs.append(oe[: splits[e]])
    full = np.concatenate(outs, axis=0).astype(x.dtype)
    return full, res.exec_time_ns


def kernel(input, weights, splits_cpu):
    out, _ = run(input, weights, splits_cpu, trace=False)
    return out

